# revision 1
# baseline (speedup 1.0000x reference)
"""Cross-attention kernel for Trainium2 (Bass/Tile), SPMD over 8 NeuronCores.

Reference computation (per batch b, all fp32):
    q = x1 @ Wq ; k = x2 @ Wk ; v = x2 @ Wv          # [S, D] each
    scores = q.T @ k                                  # [D, D], contracts S
    A = softmax(scores / 32, axis=-1)
    out = v @ A                                       # [S, D]

Algebraic refactor (saves ~1.8x FLOPs and keeps every big GEMM in the
hardware-natural layout):
    G      = x1.T @ x2            # [D, D], contracts S  (natural layouts)
    scores = Wq.T @ (G @ Wk)      # two 1024^3 GEMMs
    out    = x2 @ (Wv @ A)        # one 1024^3 GEMM + one big GEMM

Matmul operands use float32r (TF32-like rounding, fp32 accumulate): 1
cycle/row on the PE at N>=256 vs 4 cycles/row for float32. End-to-end
absmax error vs the fp32 reference is ~5e-4 of output scale.

Sharding: data-parallel over batch B=8 -> one batch element per core,
weights replicated. No collectives needed.
"""

import numpy as np

import concourse.bass as bass
import concourse.mybir as mybir
import concourse.tile as tile
from concourse import bacc
from concourse.bass_utils import run_bass_kernel_spmd
from concourse.masks import make_identity

B, S, D = 8, 4096, 1024
FP32 = mybir.dt.float32
FP32R = mybir.dt.float32r
SCALE = 1.0 / 32.0  # 1/sqrt(1024)
NS = S // 128   # 32 sequence tiles
ND = D // 128   # 8 feature tiles
NE = D // 512   # 2 free-dim halves (matmul N<=512 fp32/fp32r)


def _build(reps=1, use_f32r=True):
    nc = bacc.Bacc()
    MMDT = FP32R if use_f32r else FP32
    x1 = nc.dram_tensor("x1", [S, D], FP32, kind="ExternalInput")
    x2 = nc.dram_tensor("x2", [S, D], FP32, kind="ExternalInput")
    wq = nc.dram_tensor("wq", [D, D], FP32, kind="ExternalInput")
    wk = nc.dram_tensor("wk", [D, D], FP32, kind="ExternalInput")
    wv = nc.dram_tensor("wv", [D, D], FP32, kind="ExternalInput")
    out = nc.dram_tensor("out", [S, D], FP32, kind="ExternalOutput")

    def load(dst, src_ap):
        # fp32 -> MMDT rounding happens in the DMA (SWDGE) when needed
        if dst.dtype == FP32:
            nc.sync.dma_start(out=dst, in_=src_ap)
        else:
            nc.gpsimd.dma_start(out=dst, in_=src_ap)

    with tile.TileContext(nc) as tc:
        with (
            tc.tile_pool(name="big", bufs=28) as big,      # [128,1024] slots
            tc.tile_pool(name="xs", bufs=6) as xs,         # streamed x tiles
            tc.tile_pool(name="xh", bufs=6) as xh,         # streamed x half tiles
            tc.tile_pool(name="xt", bufs=6) as xtp,        # transposed x tiles
            tc.tile_pool(name="ob", bufs=3) as obp,        # output staging
            tc.tile_pool(name="st", bufs=16) as st,        # [128,1] stats
            tc.tile_pool(name="const", bufs=1) as cst,
            tc.tile_pool(name="ps", bufs=8, space="PSUM") as psp,
        ):
            ident = cst.tile([128, 128], FP32)
            make_identity(nc, ident[:])
            identr = cst.tile([128, 128], MMDT)
            nc.vector.tensor_copy(identr[:], ident[:])

            for _rep in range(reps):
                # ---- Phase 1: Gt[p, q] = x2.T @ x1  (contract s) ----
                # PSUM holds 8 p-tiles x [128, 512(q-half)]; two passes over q.
                gts = [big.tile([128, D], MMDT, tag="big", name=f"gt{i}")
                       for i in range(ND)]
                for qh in range(NE):
                    accs = [psp.tile([128, 512], FP32, tag="ps",
                                     name=f"acc{qh}_{i}") for i in range(ND)]
                    for si in range(NS):
                        x2t = xs.tile([128, D], MMDT, tag="xs")
                        load(x2t[:], x2[si * 128:(si + 1) * 128, :])
                        x1t = xh.tile([128, 512], MMDT, tag="xh")
                        load(x1t[:],
                             x1[si * 128:(si + 1) * 128, qh * 512:(qh + 1) * 512])
                        for p in range(ND):
                            nc.tensor.matmul(
                                accs[p][:],
                                lhsT=x2t[:, p * 128:(p + 1) * 128],
                                rhs=x1t[:],
                                start=(si == 0),
                                stop=(si == NS - 1),
                            )
                    for p in range(ND):
                        nc.vector.tensor_copy(
                            gts[p][:, qh * 512:(qh + 1) * 512], accs[p][:])

                # ---- Phase 2: T2[q, e] = Gt.T @ Wk  (contract p) ----
                wks = [big.tile([128, D], MMDT, tag="big", name=f"wk{i}")
                       for i in range(ND)]
                for p in range(ND):
                    load(wks[p][:], wk[p * 128:(p + 1) * 128, :])
                t2s = [big.tile([128, D], MMDT, tag="big", name=f"t2{i}")
                       for i in range(ND)]
                for q in range(ND):
                    for eh in range(NE):
                        ps = psp.tile([128, 512], FP32, tag="ps", name="mmps")
                        for p in range(ND):
                            nc.tensor.matmul(
                                ps[:],
                                lhsT=gts[p][:, q * 128:(q + 1) * 128],
                                rhs=wks[p][:, eh * 512:(eh + 1) * 512],
                                start=(p == 0),
                                stop=(p == ND - 1),
                            )
                        nc.vector.tensor_copy(
                            t2s[q][:, eh * 512:(eh + 1) * 512], ps[:])

                # ---- Phase 5 (hoisted): WvT[d, p] = Wv.T via PE transposes
                # 4 transposes per PSUM bank; PE work overlaps phase 3's
                # softmax bubbles.
                wvs = [big.tile([128, D], MMDT, tag="big", name=f"wv{i}")
                       for i in range(ND)]
                for p in range(ND):
                    load(wvs[p][:], wv[p * 128:(p + 1) * 128, :])
                wvts = [big.tile([128, D], MMDT, tag="big", name=f"wvt{i}")
                        for i in range(ND)]
                for d in range(ND):
                    for pg in range(2):
                        tp = psp.tile([128, 512], MMDT, tag="ps", name="tpwv")
                        for k in range(4):
                            p = pg * 4 + k
                            nc.tensor.transpose(
                                tp[:, k * 128:(k + 1) * 128],
                                wvs[p][:, d * 128:(d + 1) * 128], identr[:])
                        nc.vector.tensor_copy(
                            wvts[d][:, pg * 512:(pg + 1) * 512], tp[:])

                # ---- Phase 3+4: scores[d, e] = Wq.T @ T2, then row softmax ----
                wqs = [big.tile([128, D], MMDT, tag="big", name=f"wq{i}")
                       for i in range(ND)]
                for q in range(ND):
                    load(wqs[q][:], wq[q * 128:(q + 1) * 128, :])
                ats = [big.tile([128, D], MMDT, tag="big", name=f"at{i}")
                       for i in range(ND)]
                for d in range(ND):
                    pss = []
                    for eh in range(NE):
                        ps = psp.tile([128, 512], FP32, tag="ps",
                                      name=f"sc{d}_{eh}")
                        for q in range(ND):
                            nc.tensor.matmul(
                                ps[:],
                                lhsT=wqs[q][:, d * 128:(d + 1) * 128],
                                rhs=t2s[q][:, eh * 512:(eh + 1) * 512],
                                start=(q == 0),
                                stop=(q == ND - 1),
                            )
                        pss.append(ps)
                    mxs = []
                    for eh in range(NE):
                        mx = st.tile([128, 1], FP32, tag="st", name=f"mx{eh}")
                        nc.vector.reduce_max(
                            mx[:], pss[eh][:], axis=mybir.AxisListType.X)
                        mxs.append(mx)
                    mx = st.tile([128, 1], FP32, tag="st")
                    nc.vector.tensor_max(mx[:], mxs[0][:], mxs[1][:])
                    nb = st.tile([128, 1], FP32, tag="st")
                    nc.scalar.mul(nb[:], mx[:], -SCALE)
                    sms = []
                    for eh in range(NE):
                        sm = st.tile([128, 1], FP32, tag="st", name=f"sm{eh}")
                        nc.scalar.activation(
                            ats[d][:, eh * 512:(eh + 1) * 512], pss[eh][:],
                            mybir.ActivationFunctionType.Exp,
                            bias=nb[:], scale=SCALE, accum_out=sm[:])
                        sms.append(sm)
                    sm = st.tile([128, 1], FP32, tag="st")
                    nc.vector.tensor_add(sm[:], sms[0][:], sms[1][:])
                    rc = st.tile([128, 1], FP32, tag="st")
                    nc.vector.reciprocal(rc[:], sm[:])
                    nc.vector.tensor_scalar_mul(ats[d][:], ats[d][:], rc[:])

                # ---- Phase 6: WvA[p, e] = Wv @ A  (contract d) ----
                wvas = [big.tile([128, D], MMDT, tag="big", name=f"wva{i}")
                        for i in range(ND)]
                for p in range(ND):
                    for eh in range(NE):
                        ps = psp.tile([128, 512], FP32, tag="ps", name="mmps")
                        for d in range(ND):
                            nc.tensor.matmul(
                                ps[:],
                                lhsT=wvts[d][:, p * 128:(p + 1) * 128],
                                rhs=ats[d][:, eh * 512:(eh + 1) * 512],
                                start=(d == 0),
                                stop=(d == ND - 1),
                            )
                        nc.vector.tensor_copy(
                            wvas[p][:, eh * 512:(eh + 1) * 512], ps[:])

                # ---- Phase 7: out[s, e] = x2 @ WvA  (contract p) ----
                # Software-pipelined: transposes of s-tile si+1 are emitted
                # before the matmuls of s-tile si, so the PSUM->SBUF copies
                # of the transposed blocks hide under the 16-matmul block.
                def p7_transpose_block(si):
                    x2t = xs.tile([128, D], MMDT, tag="xs", name="x2t7")
                    load(x2t[:], x2[si * 128:(si + 1) * 128, :])
                    xts = []
                    for pg in range(2):
                        tp = psp.tile([128, 512], MMDT, tag="ps", name="tpx")
                        for k in range(4):
                            p = pg * 4 + k
                            nc.tensor.transpose(
                                tp[:, k * 128:(k + 1) * 128],
                                x2t[:, p * 128:(p + 1) * 128], identr[:])
                        xt = xtp.tile([128, 512], MMDT, tag="xt", name="xt7")
                        nc.vector.tensor_copy(xt[:], tp[:])
                        xts.append(xt)
                    return xts

                xts_cur = p7_transpose_block(0)
                for si in range(NS):
                    xts_next = (p7_transpose_block(si + 1)
                                if si + 1 < NS else None)
                    ob = obp.tile([128, D], FP32, tag="ob")
                    for eh in range(NE):
                        ps = psp.tile([128, 512], FP32, tag="ps", name="mmps")
                        for p in range(ND):
                            nc.tensor.matmul(
                                ps[:],
                                lhsT=xts_cur[p // 4][:, (p % 4) * 128:(p % 4 + 1) * 128],
                                rhs=wvas[p][:, eh * 512:(eh + 1) * 512],
                                start=(p == 0),
                                stop=(p == ND - 1),
                            )
                        nc.vector.tensor_copy(ob[:, eh * 512:(eh + 1) * 512], ps[:])
                    nc.sync.dma_start(
                        out=out[si * 128:(si + 1) * 128, :], in_=ob[:])
                    xts_cur = xts_next

    nc.finalize()
    return nc


_NC = None


def _get_nc():
    global _NC
    if _NC is None:
        _NC = _build()
    return _NC


def kernel(x_1, x_2, W_query, W_key, W_value, _results_hook=None):
    nc = _get_nc()
    x_1 = np.asarray(x_1, dtype=np.float32)
    x_2 = np.asarray(x_2, dtype=np.float32)
    wq = np.ascontiguousarray(np.asarray(W_query, dtype=np.float32))
    wk = np.ascontiguousarray(np.asarray(W_key, dtype=np.float32))
    wv = np.ascontiguousarray(np.asarray(W_value, dtype=np.float32))
    in_maps = [
        {
            "x1": np.ascontiguousarray(x_1[b]),
            "x2": np.ascontiguousarray(x_2[b]),
            "wq": wq,
            "wk": wk,
            "wv": wv,
        }
        for b in range(B)
    ]
    res = run_bass_kernel_spmd(nc, in_maps, list(range(B)))
    if _results_hook is not None:
        _results_hook(res)
    return np.stack([res.results[b]["out"] for b in range(B)], axis=0)

